# revision 5
# baseline (speedup 1.0000x reference)
"""Trainium2 Bass kernel for per-head attention (fp16 v4, software-pipelined).

Problem shapes: x [4, 1024, 12, 768]; per-head weights W_Q/K/V [12, 768, 64],
W_O [12, 64, 768]; the output projection keeps the head axis, so each of the
48 (batch, head) pairs is fully independent. Sharding: 6 pairs per core
across 8 NeuronCores (SPMD), grouped so each core sees only 2 distinct heads
(one head x 4 batches + one half-head x 2 batches) -> 2 weight DMAs per core.

All DMA-side tensors are fp16; PSUM stays fp32.

Per-pair device pipeline (x_bh [S=1024, DM=768]):
  - [kT|qT] from matmuls with packed 128-col weight chunks stationary; one
    full-width biased copy evacuates both halves ([k+bK; q+bQ]); partition-
    shift DMAs (issued from the DVE stream) replicate each half so k and q
    exist at both SBUF partition halves for row-tiled score matmuls.
  - v computed directly in [s, d] layout using xT chunks as the stationary
    operand; all 8 v-tiles built into one SBUF buffer (with interleaved
    ones-columns for the denominator trick) by a single strided copy.
  - scoresT causally chunked, two key blocks at a time via row tiling into
    the two banks of one psum tile; the causal mask is applied by an extra
    accumulating matmul (-1000 * lower-tri) on diagonal blocks, so exp
    underflow zeroes masked lanes; one Exp activation covers both blocks.
  - output projection per 128-row tile into the two banks of one psum tile;
    Wo_aug's indicator column emits the softmax denominator; one fused
    multiply-by-reciprocal evacuation per tile, split between DVE and ACT.
  - the previous pair's projection matmuls are interleaved into the current
    pair's attention phase as PE fill work (the attention phase is paced by
    the serial Exp chain on ACT; without fill the PE idles and the HAM clock
    gate re-throttles it to 1.2 GHz).
"""

import numpy as np

import concourse.bacc as bacc
import concourse.mybir as mybir
from concourse.bass_utils import run_bass_kernel_spmd
from concourse.tile import TileContext

F16 = mybir.dt.float16
F32 = mybir.dt.float32

B, S, H, DM, DH = 4, 1024, 12, 768, 64
N_CORES = 8
PAIRS_PER_CORE = (B * H) // N_CORES  # 6
MC = DM // 128  # m-chunks
ST = S // 128   # s-tiles
QC = S // 512   # q-chunks
NMASK = -1000.0  # pre-scale mask addend; exp(0.125 * -1000) == 0 exactly

# packed per-head weight blob (fp16 columns):
# [ wqk (MC*128, per-chunk [Wk|Wq]) | wv (MC*64, moving-layout W_V chunks) |
#   wo_aug (rows 0:65, 770 cols)    | bkq (f32, 2 cols: rows 0:64 = b_K,
#   rows 64:128 = b_Q) ]
WQK0, WV0 = 0, MC * 128
WO0 = WV0 + MC * DH
BKQ0 = WO0 + DM + 2
WBL = BKQ0 + 2

# DVE/ACT assignment for the 8 projection-tile evacuations
ACT_STS = (1, 3, 5, 7)
# proj tiles of the previous pair interleaved after each z-chunk (6 chunks)
FILL = (1, 1, 2, 1, 1, 2)


def _build_kernel(n_pairs=PAIRS_PER_CORE):
    nc = bacc.Bacc()

    xT = nc.declare_dram_parameter("xT", [n_pairs, DM, S], F16, isOutput=False)
    wb = nc.declare_dram_parameter("wb", [2, 128, WBL], F16, isOutput=False)
    # cmask cols 0:128 = -1000*I (stationary), 128:256 = strict-lower-tri
    cmask = nc.declare_dram_parameter("cmask", [128, 256], F16, isOutput=False)
    out = nc.declare_dram_parameter("out", [n_pairs, S, DM], F16, isOutput=True)

    with TileContext(nc) as tc:
        with (
            tc.tile_pool(name="const", bufs=1) as pconst,
            tc.tile_pool(name="xt", bufs=2) as px,
            tc.tile_pool(name="w", bufs=2) as pw,
            tc.tile_pool(name="qkv", bufs=2) as pqkv,
            tc.tile_pool(name="vaug", bufs=2) as pva,
            tc.tile_pool(name="exp", bufs=3) as pexp,
            tc.tile_pool(name="z", bufs=2) as pz,
            tc.tile_pool(name="rc", bufs=2) as prc,
            tc.tile_pool(name="outb", bufs=3) as pout,
            tc.tile_pool(name="ps_a", bufs=2, space="PSUM") as ppa,
            tc.tile_pool(name="ps_s", bufs=3, space="PSUM") as pps,
        ):
            cm = pconst.tile([128, 256], F16, name="cm")
            nc.sync.dma_start(out=cm[:], in_=cmask[:])
            negI = cm[:, 0:128]
            ltm = cm[:, 128:256]

            # PE warmup while the first x DMA is in flight (flips the HAM
            # clock gate to 8/8 before real work), plus a dummy Exp so the
            # ACT table set loads here instead of stalling the first pair.
            wscr = pconst.tile([128, 512], F16, name="wscr")
            escr = pconst.tile([1, 8], F16, name="escr")
            nc.vector.memset(wscr[:], 0.0)
            nc.scalar.activation(escr[:], wscr[0:1, 0:8],
                                 mybir.ActivationFunctionType.Exp,
                                 bias=0.0, scale=0.125)
            for wi in range(10):
                ps_w = pps.tile([128, 1024], F32, name="ps_w", tag="ps_s")
                nc.tensor.matmul(ps_w[:, 0:512], wscr[:, 0:128], wscr[:],
                                 start=True, stop=True)

            fills = []  # deferred projection-tile closures (previous pair)

            def run_fills(n):
                for _ in range(min(n, len(fills))):
                    fills.pop(0)()

            p = 0
            for g, gsize in ((0, 4), (1, 2)):
                wb_t = pw.tile([128, WBL], F16, name="wb_t", tag="wb")
                nc.sync.dma_start(out=wb_t[:], in_=wb[g])
                wqk_t = wb_t[:, WQK0:WV0].rearrange("p (c d) -> p c d", d=128)
                wv_t = wb_t[:, WV0:WO0].rearrange("p (c d) -> p c d", d=DH)
                wo_t = wb_t[0:DH + 1, WO0:WO0 + DM + 2]
                bkq_t = wb_t[:, BKQ0:BKQ0 + 2].bitcast(F32)

                for _u in range(gsize):
                    xta = px.tile([128, MC, S], F16, name="xta", tag="xta")
                    xTv = xT[p].rearrange("(c p) s -> p c s", p=128)
                    if p == 0:
                        # fine-grained first load so the very first matmuls
                        # don't wait for the whole transfer
                        for mc in range(MC):
                            nc.sync.dma_start(
                                out=xta[:, mc, :], in_=xTv[:, mc, :])
                    else:
                        nc.sync.dma_start(out=xta[:], in_=xTv)

                    # QK projection -> [k+bK; q+bQ] in one psum; one biased
                    # full-width copy; DVE-stream shift DMAs replicate the
                    # halves (k also at 64:128, q also at 0:64).
                    kq = pqkv.tile([128, S], F16, name="kq", tag="kq")
                    qlo = pqkv.tile([DH, S], F16, name="qlo", tag="qlo")
                    khi = pqkv.tile([128, S], F16, name="khi", tag="khi")
                    for sc in range(QC):
                        ps = ppa.tile([128, 512], F32, name="ps_qk",
                                      tag="ps_a")
                        for mc in range(MC):
                            nc.tensor.matmul(
                                ps[:], wqk_t[:, mc, :],
                                xta[:, mc, sc * 512:(sc + 1) * 512],
                                start=(mc == 0), stop=(mc == MC - 1))
                        cols = slice(sc * 512, (sc + 1) * 512)
                        nc.vector.tensor_scalar(
                            kq[:, cols], ps[:], bkq_t[:], None,
                            op0=mybir.AluOpType.add)
                        nc.scalar.dma_start(
                            out=khi[DH:128, cols], in_=kq[0:DH, cols])
                        nc.scalar.dma_start(
                            out=qlo[:, cols], in_=kq[DH:128, cols])

                    # V projection with the xT chunks stationary: psum gets
                    # v directly in [s, d] layout (8 groups of 64 cols in
                    # one bank); one strided copy builds all 8 v_aug tiles
                    # (65 cols each, ones-column interleaved).
                    ps_v = ppa.tile([128, 512], F32, name="ps_v", tag="ps_a")
                    for st in range(ST):
                        for mc in range(MC):
                            nc.tensor.matmul(
                                ps_v[:, st * DH:(st + 1) * DH],
                                xta[:, mc, st * 128:(st + 1) * 128],
                                wv_t[:, mc, :],
                                start=(mc == 0), stop=(mc == MC - 1))
                    va = pva.tile([128, ST * (DH + 1)], F16, name="va",
                                  tag="va")
                    nc.vector.memset(va[:], 1.0)
                    nc.vector.tensor_copy(
                        va[:].rearrange("p (s d) -> p s d", d=DH + 1)
                        [:, :, 0:DH],
                        ps_v[:].rearrange("p (s d) -> p s d", d=DH))

                    # causal scoresT -> exp -> z, with the previous pair's
                    # projection tiles as PE fill after each z-chunk.
                    z_t = pz.tile([DH + 1, S], F16, name="z_t", tag="z")
                    fi = 0
                    for j in range(QC):
                        ps_zb = ppa.tile([128, 512], F32, name="ps_z",
                                         tag="ps_a")
                        i_max = min(ST - 1, (512 * (j + 1) - 1) // 128)
                        npair = (i_max + 1) // 2
                        pend = {}

                        def emit_pair(a, j=j, pend=pend):
                            iA, iB = 2 * a, 2 * a + 1
                            c0A = max(128 * iA, 512 * j)
                            LA = 512 * (j + 1) - c0A
                            c0B = max(128 * iB, 512 * j)
                            LB = 512 * (j + 1) - c0B
                            dA = c0A == 128 * iA
                            dB = c0B == 128 * iB
                            pp = pps.tile([128, 1024], F32, name="ps_s",
                                          tag="ps_s")
                            nc.tensor.matmul(
                                pp[:, 0:LA],
                                kq[0:DH, iA * 128:(iA + 1) * 128],
                                qlo[:, c0A:c0A + LA],
                                start=True, stop=not dA)
                            nc.tensor.matmul(
                                pp[:, 512:512 + LB],
                                khi[DH:128, iB * 128:(iB + 1) * 128],
                                kq[DH:128, c0B:c0B + LB],
                                start=True, stop=not dB)
                            if dA:
                                nc.tensor.matmul(pp[:, 0:128], negI, ltm,
                                                 start=False, stop=True)
                            if dB:
                                nc.tensor.matmul(pp[:, 512:640], negI, ltm,
                                                 start=False, stop=True)
                            pend[a] = (pp, (iA, c0A, LA), (iB, c0B, LB))

                        emit_pair(0)
                        if npair > 1:
                            emit_pair(1)
                        for a in range(npair):
                            pp, (iA, c0A, LA), (iB, c0B, LB) = pend.pop(a)
                            ex = pexp.tile([128, 1024], F16, name="ex",
                                           tag="ex")
                            nc.scalar.activation(
                                ex[:, 0:512 + LB], pp[:, 0:512 + LB],
                                mybir.ActivationFunctionType.Exp,
                                bias=0.0, scale=0.125)
                            if a + 2 < npair:
                                emit_pair(a + 2)
                            nc.tensor.matmul(
                                ps_zb[0:DH + 1, c0A - 512 * j:512],
                                va[:, iA * (DH + 1):(iA + 1) * (DH + 1)],
                                ex[:, 0:LA],
                                start=(iA == 0), stop=False)
                            nc.tensor.matmul(
                                ps_zb[0:DH + 1, c0B - 512 * j:512],
                                va[:, iB * (DH + 1):(iB + 1) * (DH + 1)],
                                ex[:, 512:512 + LB],
                                start=False, stop=(iB == i_max))
                            run_fills(FILL[fi])
                            fi += 1
                        nc.vector.tensor_copy(
                            z_t[:, j * 512:(j + 1) * 512],
                            ps_zb[0:DH + 1, :])

                    # build this pair's projection-tile closures; they run
                    # as fill inside the NEXT pair's attention phase (or
                    # right away for the final pair).
                    obh_box = [None]

                    def mk_proj(st, p=p, z_t=z_t, wo_t=wo_t,
                                obh_box=obh_box, last=(p == n_pairs - 1)):
                        def run():
                            zsl = z_t[:, st * 128:(st + 1) * 128]
                            gg = st % 4
                            if gg == 0:
                                obh_box[0] = pout.tile(
                                    [128, 4, DM], F16, name="obh", tag="obh")
                            obh = obh_box[0]
                            pp = pps.tile([128, 1024], F32, name="ps_p",
                                          tag="ps_s")
                            nc.tensor.matmul(
                                pp[:, 0:384], zsl, wo_t[:, 0:384],
                                start=True, stop=True)
                            nc.tensor.matmul(
                                pp[:, 512:898], zsl, wo_t[:, 384:DM + 2],
                                start=True, stop=True)
                            rc = prc.tile([128, 1], F32, name=f"rc{st}",
                                          tag=f"rc{st}")
                            nc.vector.reciprocal(rc[:], pp[:, 896:897])
                            in_ap = pp[:].rearrange(
                                "p (b c) -> p b c", c=512)[:, :, 0:384]
                            out_ap = obh[:, gg, :].rearrange(
                                "p (b c) -> p b c", c=384)
                            if st in ACT_STS:
                                nc.scalar.mul(out_ap, in_ap, rc[:])
                            else:
                                nc.vector.tensor_scalar(
                                    out_ap, in_ap, rc[:], None,
                                    op0=mybir.AluOpType.mult)
                            if last and gg % 2 == 1:
                                nc.gpsimd.dma_start(
                                    out=out[p, (st - 1) * 128:
                                            (st + 1) * 128, :]
                                    .rearrange("(g sp) m -> sp g m", sp=128),
                                    in_=obh[:, gg - 1:gg + 1, :])
                            elif not last and gg == 3:
                                nc.gpsimd.dma_start(
                                    out=out[p, (st - 3) * 128:
                                            (st + 1) * 128, :]
                                    .rearrange("(g sp) m -> sp g m", sp=128),
                                    in_=obh[:])
                        return run

                    assert not fills
                    fills.extend(mk_proj(st) for st in range(ST))
                    if p == n_pairs - 1:
                        run_fills(ST)
                    p += 1

    nc.finalize()
    return nc


_NC_CACHE = {}


def _get_nc():
    if "nc" not in _NC_CACHE:
        _NC_CACHE["nc"] = _build_kernel()
    return _NC_CACHE["nc"]


def _core_pairs(c):
    """6 (batch, head) pairs for core c: head c x batches 0..3, plus half of
    head 8 + c//2 (2 batches)."""
    pairs = [(b, c) for b in range(B)]
    h2 = 8 + c // 2
    b0 = (c % 2) * 2
    pairs += [(b0, h2), (b0 + 1, h2)]
    return pairs


def _head_blob(W_Q, b_Q, W_K, b_K, W_V, b_V, W_O, b_O, h):
    wbh = np.zeros((128, WBL), np.float16)
    wqk = wbh[:, WQK0:WV0].reshape(128, MC, 128)
    wqk[:, :, 0:DH] = W_K[h].reshape(MC, 128, DH).transpose(1, 0, 2)
    wqk[:, :, DH:128] = W_Q[h].reshape(MC, 128, DH).transpose(1, 0, 2)
    wbh[:, WV0:WO0].reshape(128, MC, DH)[:] = \
        W_V[h].reshape(MC, 128, DH).transpose(1, 0, 2)
    wbh[0:DH, WO0:WO0 + DM] = W_O[h]
    wbh[DH, WO0:WO0 + DM] = b_V[h] @ W_O[h] + b_O / H
    wbh[DH, WO0 + DM] = 1.0
    bkq = np.concatenate([np.asarray(b_K[h], np.float32),
                          np.asarray(b_Q[h], np.float32)])
    wbh[:, BKQ0:BKQ0 + 2] = \
        np.ascontiguousarray(bkq).view(np.float16).reshape(128, 2)
    return wbh


def _make_core_inputs(x, W_Q, b_Q, W_K, b_K, W_V, b_V, W_O, b_O, c):
    pairs = _core_pairs(c)
    m = {
        "xT": np.empty((PAIRS_PER_CORE, DM, S), np.float16),
        "wb": np.empty((2, 128, WBL), np.float16),
    }
    for idx, (b, h) in enumerate(pairs):
        m["xT"][idx] = x[b, :, h, :].T
    args = (W_Q, b_Q, W_K, b_K, W_V, b_V, W_O, b_O)
    m["wb"][0] = _head_blob(*args, pairs[0][1])
    m["wb"][1] = _head_blob(*args, pairs[4][1])
    cm = np.zeros((128, 256), np.float16)
    cm[:, 0:128] = NMASK * np.eye(128, dtype=np.float16)
    ql = np.arange(128)
    cm[:, 128:256] = (ql[None, :] < ql[:, None]).astype(np.float16)
    m["cmask"] = cm
    return m


def kernel(normalized_resid_pre, W_Q, b_Q, W_K, b_K, W_V, b_V, W_O, b_O):
    x = np.ascontiguousarray(np.asarray(normalized_resid_pre, dtype=np.float32))
    args = tuple(np.asarray(a, dtype=np.float32)
                 for a in (W_Q, b_Q, W_K, b_K, W_V, b_V, W_O, b_O))

    nc = _get_nc()
    in_maps = [_make_core_inputs(x, *args, c) for c in range(N_CORES)]
    res = run_bass_kernel_spmd(nc, in_maps, list(range(N_CORES)))

    got = np.empty((B, S, H, DM), np.float32)
    for c in range(N_CORES):
        ro = np.asarray(res.results[c]["out"], np.float32)
        for idx, (b, h) in enumerate(_core_pairs(c)):
            got[b, :, h, :] = ro[idx]
    return got


# revision 10
# speedup vs baseline: 1.1713x; 1.1713x over previous
"""Trainium2 Bass kernel for per-head attention (fp16 v4, software-pipelined).

Problem shapes: x [4, 1024, 12, 768]; per-head weights W_Q/K/V [12, 768, 64],
W_O [12, 64, 768]; the output projection keeps the head axis, so each of the
48 (batch, head) pairs is fully independent. Sharding: 6 pairs per core
across 8 NeuronCores (SPMD), grouped so each core sees only 2 distinct heads
(one head x 4 batches + one half-head x 2 batches) -> 2 weight DMAs per core.

All DMA-side tensors are fp16; PSUM stays fp32.

Per-pair device pipeline (x_bh [S=1024, DM=768]):
  - [kT|qT] from matmuls with packed 128-col weight chunks stationary; one
    full-width biased copy evacuates both halves ([k+bK; q+bQ]); partition-
    shift DMAs (issued from the DVE stream) replicate each half so k and q
    exist at both SBUF partition halves for row-tiled score matmuls.
  - v computed directly in [s, d] layout using xT chunks as the stationary
    operand; all 8 v-tiles built into one SBUF buffer (with interleaved
    ones-columns for the denominator trick) by a single strided copy.
  - scoresT causally chunked, two key blocks at a time via row tiling into
    the two banks of one psum tile; the causal mask is applied by an extra
    accumulating matmul (-1000 * lower-tri) on diagonal blocks, so exp
    underflow zeroes masked lanes; one Exp activation covers both blocks.
  - output projection per 128-row tile into the two banks of one psum tile;
    Wo_aug's indicator column emits the softmax denominator; one fused
    multiply-by-reciprocal evacuation per tile, split between DVE and ACT.
  - the previous pair's projection matmuls are interleaved into the current
    pair's attention phase as PE fill work (the attention phase is paced by
    the serial Exp chain on ACT; without fill the PE idles and the HAM clock
    gate re-throttles it to 1.2 GHz).
"""

import numpy as np

import concourse.bacc as bacc
import concourse.mybir as mybir
from concourse.bass_utils import run_bass_kernel_spmd
from concourse.tile import TileContext

F16 = mybir.dt.float16
F32 = mybir.dt.float32

B, S, H, DM, DH = 4, 1024, 12, 768, 64
N_CORES = 8
PAIRS_PER_CORE = (B * H) // N_CORES  # 6
MC = DM // 128  # m-chunks
ST = S // 128   # s-tiles
QC = S // 512   # q-chunks
NMASK = -1000.0  # pre-scale mask addend; exp(0.125 * -1000) == 0 exactly

# packed per-head weight blob (fp16 columns):
# [ wqk (MC*128, per-chunk [Wk|Wq]) | wv (MC*64, moving-layout W_V chunks) |
#   wo_aug (rows 0:65, 770 cols)    | bkq (f32, 2 cols: rows 0:64 = b_K,
#   rows 64:128 = b_Q) ]
WQK0, WV0 = 0, MC * 128
WO0 = WV0 + MC * DH
BKQ0 = WO0 + DM + 2
WBL = BKQ0 + 2

# DVE/ACT assignment for the 8 projection-tile evacuations
ACT_STS = (1, 3, 5, 7)


def _build_kernel(n_pairs=PAIRS_PER_CORE):
    nc = bacc.Bacc()

    xT = nc.declare_dram_parameter("xT", [n_pairs, DM, S], F16, isOutput=False)
    wb = nc.declare_dram_parameter("wb", [2, 128, WBL], F16, isOutput=False)
    # cmask cols 0:128 = -1000*I (stationary), 128:256 = strict-lower-tri
    cmask = nc.declare_dram_parameter("cmask", [128, 256], F16, isOutput=False)
    out = nc.declare_dram_parameter("out", [n_pairs, S, DM], F16, isOutput=True)

    with TileContext(nc) as tc:
        with (
            tc.tile_pool(name="const", bufs=1) as pconst,
            tc.tile_pool(name="xt", bufs=2) as px,
            tc.tile_pool(name="w", bufs=2) as pw,
            tc.tile_pool(name="qkv", bufs=2) as pqkv,
            tc.tile_pool(name="vaug", bufs=2) as pva,
            tc.tile_pool(name="exp", bufs=3) as pexp,
            tc.tile_pool(name="z", bufs=2) as pz,
            tc.tile_pool(name="rc", bufs=2) as prc,
            tc.tile_pool(name="outb", bufs=3) as pout,
            tc.tile_pool(name="ps_a", bufs=2, space="PSUM") as ppa,
            tc.tile_pool(name="ps_s", bufs=2, space="PSUM") as pps,
            tc.tile_pool(name="ps_p", bufs=1, space="PSUM") as ppp,
        ):
            cm = pconst.tile([128, 256], F16, name="cm")
            nc.sync.dma_start(out=cm[:], in_=cmask[:])
            negI = cm[:, 0:128]
            ltm = cm[:, 128:256]

            # PE warmup while the first x DMA is in flight (flips the HAM
            # clock gate to 8/8 before real work), plus a dummy Exp so the
            # ACT table set loads here instead of stalling the first pair.
            wscr = pconst.tile([128, 512], F16, name="wscr")
            escr = pconst.tile([1, 8], F16, name="escr")
            nc.vector.memset(wscr[:], 0.0)
            nc.scalar.activation(escr[:], wscr[0:1, 0:8],
                                 mybir.ActivationFunctionType.Exp,
                                 bias=0.0, scale=0.125)
            for wi in range(10):
                ps_w = pps.tile([128, 1024], F32, name="ps_w", tag="ps_s")
                nc.tensor.matmul(ps_w[:, 0:512], wscr[:, 0:128], wscr[:],
                                 start=True, stop=True)

            fills = []  # deferred projection-tile closures (previous pair)

            def run_fills(n):
                for _ in range(min(n, len(fills))):
                    fills.pop(0)()

            p = 0
            for g, gsize in ((0, 4), (1, 2)):
                wb_t = pw.tile([128, WBL], F16, name="wb_t", tag="wb")
                nc.sync.dma_start(out=wb_t[:], in_=wb[g])
                wqk_t = wb_t[:, WQK0:WV0].rearrange("p (c d) -> p c d", d=128)
                wv_t = wb_t[:, WV0:WO0].rearrange("p (c d) -> p c d", d=DH)
                wo_t = wb_t[0:DH + 1, WO0:WO0 + DM + 2]
                bkq_t = wb_t[:, BKQ0:BKQ0 + 2].bitcast(F32)

                for _u in range(gsize):
                    xta = px.tile([128, MC, S], F16, name="xta", tag="xta")
                    xTv = xT[p].rearrange("(c p) s -> p c s", p=128)
                    if p == 0:
                        # fine-grained first load so the very first matmuls
                        # don't wait for the whole transfer
                        for mc in range(MC):
                            nc.sync.dma_start(
                                out=xta[:, mc, :], in_=xTv[:, mc, :])
                    else:
                        nc.sync.dma_start(out=xta[:], in_=xTv)

                    # QK projection -> [k+bK; q+bQ] in one psum; one biased
                    # full-width copy; DVE-stream shift DMAs replicate the
                    # halves (k also at 64:128, q also at 0:64).
                    kq = pqkv.tile([128, S], F16, name="kq", tag="kq")
                    qlo = pqkv.tile([DH, S], F16, name="qlo", tag="qlo")
                    khi = pqkv.tile([128, S], F16, name="khi", tag="khi")
                    for sc in range(QC):
                        ps = ppa.tile([128, 512], F32, name="ps_qk",
                                      tag="ps_a")
                        for mc in range(MC):
                            nc.tensor.matmul(
                                ps[:], wqk_t[:, mc, :],
                                xta[:, mc, sc * 512:(sc + 1) * 512],
                                start=(mc == 0), stop=(mc == MC - 1))
                        cols = slice(sc * 512, (sc + 1) * 512)
                        nc.vector.tensor_scalar(
                            kq[:, cols], ps[:], bkq_t[:], None,
                            op0=mybir.AluOpType.add)
                        nc.sync.dma_start(
                            out=khi[DH:128, cols], in_=kq[0:DH, cols])
                        nc.sync.dma_start(
                            out=qlo[:, cols], in_=kq[DH:128, cols])
                        run_fills(1)

                    # V projection with the xT chunks stationary: psum gets
                    # v directly in [s, d] layout (8 groups of 64 cols in
                    # one bank); one strided copy builds all 8 v_aug tiles
                    # (65 cols each, ones-column interleaved).
                    ps_v = ppa.tile([128, 512], F32, name="ps_v", tag="ps_a")
                    for st in range(ST):
                        for mc in range(MC):
                            nc.tensor.matmul(
                                ps_v[:, st * DH:(st + 1) * DH],
                                xta[:, mc, st * 128:(st + 1) * 128],
                                wv_t[:, mc, :],
                                start=(mc == 0), stop=(mc == MC - 1))
                    va = pva.tile([128, ST * (DH + 1)], F16, name="va",
                                  tag="va")
                    nc.vector.memset(va[:], 1.0)
                    nc.vector.tensor_copy(
                        va[:].rearrange("p (s d) -> p s d", d=DH + 1)
                        [:, :, 0:DH],
                        ps_v[:].rearrange("p (s d) -> p s d", d=DH))

                    # causal scoresT -> exp -> z, with the previous pair's
                    # projection tiles as PE fill after each z-chunk.
                    z_t = pz.tile([DH + 1, S], F16, name="z_t", tag="z")
                    fi = 0
                    for j in range(QC):
                        ps_zb = ppa.tile([128, 512], F32, name="ps_z",
                                         tag="ps_a")
                        i_max = min(ST - 1, (512 * (j + 1) - 1) // 128)
                        npair = (i_max + 1) // 2
                        pend = {}

                        def emit_pair(a, j=j, pend=pend):
                            iA, iB = 2 * a, 2 * a + 1
                            c0A = max(128 * iA, 512 * j)
                            LA = 512 * (j + 1) - c0A
                            c0B = max(128 * iB, 512 * j)
                            LB = 512 * (j + 1) - c0B
                            dA = c0A == 128 * iA
                            dB = c0B == 128 * iB
                            pp = pps.tile([128, 1024], F32, name="ps_s",
                                          tag="ps_s")
                            nc.tensor.matmul(
                                pp[:, 0:LA],
                                kq[0:DH, iA * 128:(iA + 1) * 128],
                                qlo[:, c0A:c0A + LA],
                                start=True, stop=not dA)
                            nc.tensor.matmul(
                                pp[:, 512:512 + LB],
                                khi[DH:128, iB * 128:(iB + 1) * 128],
                                kq[DH:128, c0B:c0B + LB],
                                start=True, stop=not dB)
                            if dA:
                                nc.tensor.matmul(pp[:, 0:128], negI, ltm,
                                                 start=False, stop=True)
                            if dB:
                                nc.tensor.matmul(pp[:, 512:640], negI, ltm,
                                                 start=False, stop=True)
                            pend[a] = (pp, (iA, c0A, LA), (iB, c0B, LB))

                        emit_pair(0)
                        if npair > 1:
                            emit_pair(1)
                        for a in range(npair):
                            pp, (iA, c0A, LA), (iB, c0B, LB) = pend.pop(a)
                            ex = pexp.tile([128, 1024], F16, name="ex",
                                           tag="ex")
                            nc.scalar.activation(
                                ex[:, 0:512 + LB], pp[:, 0:512 + LB],
                                mybir.ActivationFunctionType.Exp,
                                bias=0.0, scale=0.125)
                            if a + 2 < npair:
                                emit_pair(a + 2)
                            nc.tensor.matmul(
                                ps_zb[0:DH + 1, c0A - 512 * j:512],
                                va[:, iA * (DH + 1):(iA + 1) * (DH + 1)],
                                ex[:, 0:LA],
                                start=(iA == 0), stop=False)
                            nc.tensor.matmul(
                                ps_zb[0:DH + 1, c0B - 512 * j:512],
                                va[:, iB * (DH + 1):(iB + 1) * (DH + 1)],
                                ex[:, 512:512 + LB],
                                start=False, stop=(iB == i_max))
                            run_fills(1)
                            fi += 1
                        nc.vector.tensor_copy(
                            z_t[:, j * 512:(j + 1) * 512],
                            ps_zb[0:DH + 1, :])

                    # build this pair's projection-tile closures; they run
                    # as fill inside the NEXT pair's attention phase (or
                    # right away for the final pair).
                    obh_box = [None]

                    def mk_proj(st, p=p, z_t=z_t, wo_t=wo_t,
                                obh_box=obh_box, last=(p == n_pairs - 1)):
                        def run():
                            zsl = z_t[:, st * 128:(st + 1) * 128]
                            gg = st % 4
                            if gg == 0:
                                obh_box[0] = pout.tile(
                                    [128, 4, DM], F16, name="obh", tag="obh")
                            obh = obh_box[0]
                            pp = ppp.tile([128, 1024], F32, name="ps_p",
                                          tag="ps_p")
                            nc.tensor.matmul(
                                pp[:, 0:384], zsl, wo_t[:, 0:384],
                                start=True, stop=True)
                            nc.tensor.matmul(
                                pp[:, 512:898], zsl, wo_t[:, 384:DM + 2],
                                start=True, stop=True)
                            rc = prc.tile([128, 1], F32, name=f"rc{st}",
                                          tag=f"rc{st}")
                            nc.vector.reciprocal(rc[:], pp[:, 896:897])
                            in_ap = pp[:].rearrange(
                                "p (b c) -> p b c", c=512)[:, :, 0:384]
                            out_ap = obh[:, gg, :].rearrange(
                                "p (b c) -> p b c", c=384)
                            if st in ACT_STS:
                                nc.scalar.mul(out_ap, in_ap, rc[:])
                            else:
                                nc.vector.tensor_scalar(
                                    out_ap, in_ap, rc[:], None,
                                    op0=mybir.AluOpType.mult)
                            if last and gg % 2 == 1:
                                nc.gpsimd.dma_start(
                                    out=out[p, (st - 1) * 128:
                                            (st + 1) * 128, :]
                                    .rearrange("(g sp) m -> sp g m", sp=128),
                                    in_=obh[:, gg - 1:gg + 1, :])
                            elif not last and gg == 3:
                                nc.gpsimd.dma_start(
                                    out=out[p, (st - 3) * 128:
                                            (st + 1) * 128, :]
                                    .rearrange("(g sp) m -> sp g m", sp=128),
                                    in_=obh[:])
                        return run

                    assert not fills
                    fills.extend(mk_proj(st) for st in range(ST))
                    if p == n_pairs - 1:
                        run_fills(ST)
                    p += 1

    nc.finalize()
    return nc


_NC_CACHE = {}


def _get_nc():
    if "nc" not in _NC_CACHE:
        _NC_CACHE["nc"] = _build_kernel()
    return _NC_CACHE["nc"]


def _core_pairs(c):
    """6 (batch, head) pairs for core c: head c x batches 0..3, plus half of
    head 8 + c//2 (2 batches)."""
    pairs = [(b, c) for b in range(B)]
    h2 = 8 + c // 2
    b0 = (c % 2) * 2
    pairs += [(b0, h2), (b0 + 1, h2)]
    return pairs


def _head_blob(W_Q, b_Q, W_K, b_K, W_V, b_V, W_O, b_O, h):
    wbh = np.zeros((128, WBL), np.float16)
    wqk = wbh[:, WQK0:WV0].reshape(128, MC, 128)
    wqk[:, :, 0:DH] = W_K[h].reshape(MC, 128, DH).transpose(1, 0, 2)
    wqk[:, :, DH:128] = W_Q[h].reshape(MC, 128, DH).transpose(1, 0, 2)
    wbh[:, WV0:WO0].reshape(128, MC, DH)[:] = \
        W_V[h].reshape(MC, 128, DH).transpose(1, 0, 2)
    wbh[0:DH, WO0:WO0 + DM] = W_O[h]
    wbh[DH, WO0:WO0 + DM] = b_V[h] @ W_O[h] + b_O / H
    wbh[DH, WO0 + DM] = 1.0
    bkq = np.concatenate([np.asarray(b_K[h], np.float32),
                          np.asarray(b_Q[h], np.float32)])
    wbh[:, BKQ0:BKQ0 + 2] = \
        np.ascontiguousarray(bkq).view(np.float16).reshape(128, 2)
    return wbh


def _make_core_inputs(x, W_Q, b_Q, W_K, b_K, W_V, b_V, W_O, b_O, c):
    pairs = _core_pairs(c)
    m = {
        "xT": np.empty((PAIRS_PER_CORE, DM, S), np.float16),
        "wb": np.empty((2, 128, WBL), np.float16),
    }
    for idx, (b, h) in enumerate(pairs):
        m["xT"][idx] = x[b, :, h, :].T
    args = (W_Q, b_Q, W_K, b_K, W_V, b_V, W_O, b_O)
    m["wb"][0] = _head_blob(*args, pairs[0][1])
    m["wb"][1] = _head_blob(*args, pairs[4][1])
    cm = np.zeros((128, 256), np.float16)
    cm[:, 0:128] = NMASK * np.eye(128, dtype=np.float16)
    ql = np.arange(128)
    cm[:, 128:256] = (ql[None, :] < ql[:, None]).astype(np.float16)
    m["cmask"] = cm
    return m


def kernel(normalized_resid_pre, W_Q, b_Q, W_K, b_K, W_V, b_V, W_O, b_O):
    x = np.ascontiguousarray(np.asarray(normalized_resid_pre, dtype=np.float32))
    args = tuple(np.asarray(a, dtype=np.float32)
                 for a in (W_Q, b_Q, W_K, b_K, W_V, b_V, W_O, b_O))

    nc = _get_nc()
    in_maps = [_make_core_inputs(x, *args, c) for c in range(N_CORES)]
    res = run_bass_kernel_spmd(nc, in_maps, list(range(N_CORES)))

    got = np.empty((B, S, H, DM), np.float32)
    for c in range(N_CORES):
        ro = np.asarray(res.results[c]["out"], np.float32)
        for idx, (b, h) in enumerate(_core_pairs(c)):
            got[b, :, h, :] = ro[idx]
    return got
